# revision 25
# baseline (speedup 1.0000x reference)
"""Trainium2 Bass kernel for nn_Encoder_47450798686673 (gnn_message_passing).

Structure exploited: the 2048 disjoint 21-node graphs all share one tiled
420-edge weight vector, so the normalized-Laplacian propagation matrix is the
SAME 21x21 matrix M for every graph.  ChebConv therefore reduces to dense
per-graph-block matmuls with host-precomputed Chebyshev matrices T_k(M):

    layer(X) = sum_k T_k(M) @ X_g @ W[k] + b      (per graph g)

Device algorithm (data-parallel over graphs, 8 cores x 256 graphs):
  - nodes are packed 6 graphs = 126 rows per block (128-partition tiles)
  - x is pre-transposed on host so features sit in partitions (contract dim)
  - per block:  C_k = X @ W1[k] (24 accumulating matmuls, weights moving —
    the FLOP floor), then the transposed mix psyT = sum_k C_k^T BD(T_k)^T
    with C_k as stationary (output lands feature-major, so relu writes H^T
    directly and layer 2 needs no transpose), then the same shape with W2
    and a natural-orientation mix2 whose T_0=I term is folded into the
    output copy as a DVE add.  Bias enters as a rank-1 matmul only when
    nonzero.  Block emission is software-pipelined (B1' of block b+1 ahead
    of the tail of block b) because engine queues are FIFO.
All accumulation is fp32 in PSUM; operand dtype is switchable.

Rejected alternatives (measured):
  - fp32 operands: exact but 4 cyc/row on PE -> 3.6x slower end to end.
  - eigendecomposition of M (folds the k-sum into per-eigenindex weights,
    5x fewer matmul FLOPs): mathematically exact in f32 (2.5e-6) but
    cond(V)~27 amplifies bf16 rounding to 1.97e-2 absmax rel err.
"""

import sys
import numpy as np

sys.path.insert(0, "/opt/trn_rl_repo")

import concourse.bass as bass  # noqa: E402,F401
import concourse.tile as tile  # noqa: E402
from concourse import bacc, mybir  # noqa: E402
from concourse import bass_utils  # noqa: E402
import ml_dtypes  # noqa: E402

# ---------------- problem constants (hardcoded per contract) ----------------
C = 21           # nodes per graph
BG = 2048        # graphs
N = C * BG       # 43008 nodes
EP = C * C - C   # 420 edges per graph
F_IN = 1024
K = 5
F1 = 256         # layer-1 out features
F2 = 64          # layer-2 out features

NCORES = 8
GPC = BG // NCORES          # 256 graphs per core
GB = 6                      # graphs per block
NB = GB * C                 # 126 nodes per block
BLOCKS = 43                 # per core: ceil(256/6) blocks (2 pad graphs)
SUPERS = 12                 # DMA super-tiles per core
SUP_BLOCKS = [1, 3] + [4] * 9 + [3]  # blocks per super (small first super)
NREAL = GPC * C             # 5376 real nodes per core
NPAD = BLOCKS * NB          # 5418 padded nodes per core
CH1 = F_IN // 128           # 8 contract chunks, layer 1
CH2 = F1 // 128             # 2 contract chunks, layer 2
W1COLS = K * F1             # 1280 per contract chunk
W2COLS = K * F2             # 320 per contract chunk
GROUPS = [(0, 512), (512, 512), (1024, 256)]   # kk-group splits of 1280
XTW = CH1 * 4 * NB          # xtb dram row width (max super: 4 blocks)

# matmul operand dtype: "f32" (exact, 4 cyc/row), "f32r" (fast, ~tf32),
# "bf16" (fast, ~4e-3 rel err)
DTYPE_MODE = "bf16"

_F32 = mybir.dt.float32
_NC_CACHE = {}

# pool-bufs tuning knobs (PSUM budget: psc + psy + pst + psc2 + pso <= 8)
BUFS = dict(xp=3, sbc=4, hp=2, htp=2, sbc2=2, op=3,
            psc=3, psyt=2, psc2=1, pso=2)
CONST_DMA = "gpsimd"   # engine for const loads: "sync" | "gpsimd"
OUT_DMA = "sync"       # engine for output stores


def _np_dt(mode):
    return ml_dtypes.bfloat16 if mode == "bf16" else np.float32


def _bir_dt(mode):
    return mybir.dt.bfloat16 if mode == "bf16" else mybir.dt.float32


def _build_nc(mode, zero_b1=True, zero_b2=True):
    """Build + compile the per-core Bass program (identical on all cores)."""
    key = (mode, zero_b1, zero_b2, tuple(sorted(BUFS.items())), CONST_DMA, OUT_DMA)
    if key in _NC_CACHE:
        return _NC_CACHE[key]
    DT = _bir_dt(mode)

    def mm(ap):
        # float32r is float32 data with a relaxed-precision matmul tag
        return ap.bitcast(mybir.dt.float32r) if mode == "f32r" else ap

    nc = bacc.Bacc("TRN2", target_bir_lowering=False, debug=False,
                   num_devices=NCORES)
    xtb = nc.dram_tensor("xtb", [SUPERS * 128, XTW], DT,
                         kind="ExternalInput").ap()
    w1s = nc.dram_tensor("w1s", [128, CH1 * W1COLS], DT,
                         kind="ExternalInput").ap()
    w2s = nc.dram_tensor("w2s", [128, CH2 * W2COLS], DT,
                         kind="ExternalInput").ap()
    bdt = nc.dram_tensor("bdt", [NB, K * NB], DT, kind="ExternalInput").ap()
    idn = nc.dram_tensor("idn", [NB, NB], DT, kind="ExternalInput").ap()
    one = nc.dram_tensor("one", [1, NB], DT, kind="ExternalInput").ap()
    bi1 = nc.dram_tensor("bi1", [1, F1], DT, kind="ExternalInput").ap()
    bi2 = nc.dram_tensor("bi2", [1, F2], DT, kind="ExternalInput").ap()
    out = nc.dram_tensor("out", [NPAD, F2], _F32, kind="ExternalOutput").ap()

    cdma = (nc.gpsimd if CONST_DMA == "gpsimd" else nc.sync).dma_start
    odma = (nc.gpsimd if OUT_DMA == "gpsimd" else nc.sync).dma_start
    with tile.TileContext(nc) as tc:
        with (
            tc.tile_pool(name="const", bufs=1) as constp,
            tc.tile_pool(name="xp", bufs=BUFS["xp"]) as xp,
            tc.tile_pool(name="sbc", bufs=BUFS["sbc"]) as sbcp,
            tc.tile_pool(name="hp", bufs=BUFS["hp"]) as hp,
            tc.tile_pool(name="htp", bufs=BUFS["htp"]) as htp,
            tc.tile_pool(name="sbc2", bufs=BUFS["sbc2"]) as sbc2p,
            tc.tile_pool(name="op", bufs=BUFS["op"]) as op,
            tc.tile_pool(name="psc", bufs=BUFS["psc"], space="PSUM") as pscp,
            tc.tile_pool(name="psyt", bufs=BUFS["psyt"], space="PSUM") as psytp,
            tc.tile_pool(name="psc2", bufs=BUFS["psc2"], space="PSUM") as psc2p,
            tc.tile_pool(name="pso", bufs=BUFS["pso"], space="PSUM") as psop,
        ):
            # small consts first (fast), then first x super, then weights
            bdtt = constp.tile([NB, K * NB], DT)
            cdma(bdtt[:], bdt[:, :])
            onet = b1t = b2t = None
            if not (zero_b1 and zero_b2):
                onet = constp.tile([1, NB], DT)
                cdma(onet[:], one[:, :])
            if not zero_b1:
                b1t = constp.tile([1, F1], DT)
                cdma(b1t[:], bi1[:, :])
            if not zero_b2:
                b2t = constp.tile([1, F2], DT)
                cdma(b2t[:], bi2[:, :])

            xts = [None] * SUPERS

            def load_super(s):
                nbk = SUP_BLOCKS[s]
                w = CH1 * nbk * NB
                t = xp.tile([128, CH1 * 4 * NB], DT, tag="xt")
                nc.sync.dma_start(t[:, :w], xtb[s * 128:(s + 1) * 128, :w])
                xts[s] = t

            load_super(0)
            # per-chunk weight tiles so the first matmuls start early
            w1t = []
            for c in range(CH1):
                t = constp.tile([128, W1COLS], DT, tag=f"w1c{c}")
                nc.sync.dma_start(t[:], w1s[:, c * W1COLS:(c + 1) * W1COLS])
                w1t.append(t)
            w2t = constp.tile([128, CH2 * W2COLS], DT)
            nc.sync.dma_start(w2t[:], w2s[:, :])
            load_super(1)

            # software pipeline: emit B1' of block b+1 before the
            # layer-2 tail of block b, so PE never stalls on the serial
            # relu/transpose/copy chain (engine queues are FIFO).
            blocks = []
            for s in range(SUPERS):
                for b in range(SUP_BLOCKS[s]):
                    blocks.append((s, b))

            def stage1(s, b):
                stride = SUP_BLOCKS[s] * NB
                xt = xts[s]
                cs = []
                for (ko, wd) in GROUPS:
                    psc = pscp.tile([NB, 512], _F32, tag="psc")
                    for c in range(CH1):
                        lhs = xt[:, c * stride + b * NB:
                                 c * stride + (b + 1) * NB]
                        rhs = w1t[c][:, ko:ko + wd]
                        nc.tensor.matmul(psc[:, :wd], mm(lhs), mm(rhs),
                                         start=(c == 0), stop=(c == CH1 - 1))
                    sbc = sbcp.tile([NB, 512], DT, tag="sbc")
                    nc.vector.tensor_copy(out=sbc[:, :wd], in_=psc[:, :wd])
                    cs.append(sbc)
                return cs

            def stage2(s, b, cs):
                # ---- transposed mix: psyT[f,i] = sum_k (C_k^T BD_k^T)[f,i]
                # C_k (nodes-part) is the stationary; BD(T_k)^T the moving.
                # Output lands feature-major, so relu writes H^T directly and
                # no PE transpose of H is needed.
                psyts = []
                for h2 in range(CH2):
                    psyt = psytp.tile([128, NB], _F32, tag="psyt")
                    for k in range(K):
                        gi, off = (k * F1) // 512, (k * F1) % 512
                        lhs = cs[gi][:, off + 128 * h2: off + 128 * (h2 + 1)]
                        nc.tensor.matmul(psyt[:], mm(lhs),
                                         mm(bdtt[:, k * NB:(k + 1) * NB]),
                                         start=(k == 0),
                                         stop=(k == K - 1 and zero_b1))
                    if not zero_b1:
                        nc.tensor.matmul(psyt[:],
                                         mm(b1t[:, 128 * h2:128 * (h2 + 1)]),
                                         mm(onet[:, :]), start=False, stop=True)
                    psyts.append(psyt)
                # ---- relu -> H^T
                ht = htp.tile([128, CH2 * NB], DT)
                for h2 in range(CH2):
                    nc.scalar.activation(ht[:, h2 * NB:(h2 + 1) * NB],
                                         psyts[h2][:],
                                         mybir.ActivationFunctionType.Relu)
                # ---- layer 2: C2_k = H @ W2[k]
                psc2 = psc2p.tile([NB, W2COLS], _F32)
                for c2 in range(CH2):
                    nc.tensor.matmul(psc2[:],
                                     mm(ht[:, c2 * NB:(c2 + 1) * NB]),
                                     mm(w2t[:, c2 * W2COLS:(c2 + 1) * W2COLS]),
                                     start=(c2 == 0), stop=(c2 == CH2 - 1))
                sbc2 = sbc2p.tile([NB, W2COLS], DT)
                nc.vector.tensor_copy(out=sbc2[:], in_=psc2[:])
                # ---- mix2: out = C2_0 + sum_{k>=1} BD(T_k) C2_k (+ b2)
                pso = psop.tile([NB, F2], _F32)
                for k in range(1, K):
                    nc.tensor.matmul(pso[:], mm(bdtt[:, k * NB:(k + 1) * NB]),
                                     mm(sbc2[:, k * F2:(k + 1) * F2]),
                                     start=(k == 1),
                                     stop=(k == K - 1 and zero_b2))
                if not zero_b2:
                    nc.tensor.matmul(pso[:], mm(onet[:, :]), mm(b2t[:, :]),
                                     start=False, stop=True)
                osb = op.tile([NB, F2], _F32)
                nc.vector.scalar_tensor_tensor(
                    out=osb[:], in0=pso[:], scalar=1.0,
                    in1=sbc2[:, :F2], op0=mybir.AluOpType.mult,
                    op1=mybir.AluOpType.add)
                nb0 = (sum(SUP_BLOCKS[:s]) + b) * NB
                odma(out[nb0:nb0 + NB, :], osb[:])

            prev = None
            for i, (s, b) in enumerate(blocks):
                if b == 0 and s + 2 < SUPERS:
                    load_super(s + 2)
                cs = stage1(s, b)
                if prev is not None:
                    stage2(*prev)
                prev = (s, b, cs)
            stage2(*prev)
    nc.compile()
    _NC_CACHE[key] = nc
    return nc


# ------------------------- host-side math -------------------------

def _adj_update_host(ew, W_fc1, W_fc2):
    h = ew.astype(np.float32).T @ W_fc1
    h = np.where(h > 0, h, np.expm1(h))           # elu
    h = np.maximum(np.tanh(h @ W_fc2), 0.0)
    return h.T.astype(np.float32)                 # (420, 1)


def _build_M(edge_index, w_full):
    """Per-graph propagation matrices from the actual edge list.

    Returns (M_all (BG,21,21) or None, norm, src, dst). None when edges
    cross graph blocks (caller must fall back)."""
    src = np.asarray(edge_index[0], dtype=np.int64)
    dst = np.asarray(edge_index[1], dtype=np.int64)
    deg = np.zeros(N, np.float32)
    np.add.at(deg, src, w_full)
    dis = np.where(deg > 0, 1.0 / np.sqrt(deg), 0.0).astype(np.float32)
    norm = -(dis[src] * w_full * dis[dst])
    if len(src) != BG * EP:
        return None, norm, src, dst
    g = np.repeat(np.arange(BG, dtype=np.int64), EP)
    sl, dl = src - g * C, dst - g * C
    if sl.min() < 0 or sl.max() >= C or dl.min() < 0 or dl.max() >= C:
        return None, norm, src, dst
    M_all = np.zeros((BG, C, C), np.float32)
    np.add.at(M_all, (g, dl, sl), norm)
    return M_all, norm, src, dst


def _cheb_T(M):
    T = [np.eye(C, dtype=np.float32), M.astype(np.float32)]
    for _ in range(2, K):
        T.append(2.0 * M @ T[-1] - T[-2])
    return np.stack(T)  # (K, 21, 21)


def _fallback_numpy(x, norm, src, dst, W1, b1, W2, b2):
    """Generic (unstructured-edge) implementation, chunked scatter-add."""
    def prop(z):
        outz = np.zeros_like(z)
        step = 1 << 17
        for e0 in range(0, len(src), step):
            sl = slice(e0, e0 + step)
            np.add.at(outz, dst[sl], norm[sl, None] * z[src[sl]])
        return outz

    def layer(z, W, b):
        t0, t1 = z, prop(z)
        o = t0 @ W[0] + t1 @ W[1]
        for k in range(2, K):
            t2 = 2.0 * prop(t1) - t0
            o += t2 @ W[k]
            t0, t1 = t1, t2
        return o + b

    h = np.maximum(layer(x, W1, b1), 0.0)
    return layer(h, W2, b2)


def _device_in_maps(x, T, W1, b1, W2, b2, mode):
    """Build per-core device input dicts from full inputs + Chebyshev T."""
    dt = _np_dt(mode)
    xtbs = []
    for i in range(NCORES):
        xc = np.zeros((NPAD, F_IN), dtype=dt)
        xc[:NREAL] = x[i * NREAL:(i + 1) * NREAL]
        slab = np.zeros((SUPERS * 128, XTW), dtype=dt)
        n0 = 0
        for s in range(SUPERS):
            ns = SUP_BLOCKS[s] * NB
            # [p, c*ns + n] = xc[n0 + n, c*128 + p]
            a = xc[n0:n0 + ns].reshape(ns, CH1, 128).transpose(2, 1, 0)
            slab[s * 128:(s + 1) * 128, :CH1 * ns] = \
                np.ascontiguousarray(a).reshape(128, CH1 * ns)
            n0 += ns
        xtbs.append(slab)

    w1s = np.ascontiguousarray(
        W1.reshape(K, CH1, 128, F1).transpose(2, 1, 0, 3)).reshape(
            128, CH1 * W1COLS).astype(dt)
    w2s = np.ascontiguousarray(
        W2.reshape(K, CH2, 128, F2).transpose(2, 1, 0, 3)).reshape(
            128, CH2 * W2COLS).astype(dt)

    bd = np.zeros((K, NB, NB), np.float32)
    for g6 in range(GB):
        sl = slice(g6 * C, (g6 + 1) * C)
        bd[:, sl, sl] = T.transpose(0, 2, 1)      # BD(T_k)^T blocks
    bdt = np.ascontiguousarray(bd.transpose(1, 0, 2)).reshape(
        NB, K * NB).astype(dt)

    common = {
        "w1s": w1s,
        "w2s": w2s,
        "bdt": bdt,
        "idn": np.eye(NB, dtype=dt),
        "one": np.ones((1, NB), dtype=dt),
        "bi1": b1.reshape(1, F1).astype(dt),
        "bi2": b2.reshape(1, F2).astype(dt),
    }
    return [dict(common, xtb=xtbs[i]) for i in range(NCORES)]


# ------------------------- entry point -------------------------

def kernel(x, edge_index, edge_weight, W_fc1, W_fc2, W1, b1, W2, b2):
    x = np.asarray(x, dtype=np.float32)
    W_fc1 = np.asarray(W_fc1, np.float32)
    W_fc2 = np.asarray(W_fc2, np.float32)
    W1 = np.asarray(W1, np.float32)
    W2 = np.asarray(W2, np.float32)
    b1 = np.asarray(b1, np.float32)
    b2 = np.asarray(b2, np.float32)

    ew = _adj_update_host(np.asarray(edge_weight, np.float32), W_fc1, W_fc2)
    reps = edge_index.shape[-1] // ew.shape[0]
    train_ew = np.tile(ew, (reps, 1))
    w_full = train_ew[:, 0]

    M_all, norm, src, dst = _build_M(edge_index, w_full)
    if M_all is None or np.abs(M_all - M_all[0]).max() > 0:
        out = _fallback_numpy(x, norm, src, dst, W1, b1, W2, b2)
        return out, ew, train_ew

    T = _cheb_T(M_all[0])
    mode = DTYPE_MODE
    in_maps = _device_in_maps(x, T, W1, b1, W2, b2, mode)
    nc = _build_nc(mode, zero_b1=not np.any(b1), zero_b2=not np.any(b2))
    res = bass_utils.run_bass_kernel_spmd(nc, in_maps,
                                          core_ids=list(range(NCORES)))

    out = np.empty((N, F2), np.float32)
    for i in range(NCORES):
        out[i * NREAL:(i + 1) * NREAL] = res.results[i]["out"][:NREAL]
    return out, ew, train_ew
